# revision 24
# baseline (speedup 1.0000x reference)
"""Causal self-attention (B=4, T=2048, C=1024, H=16) on 8 trn2 NeuronCores.

Sharding: core c -> (batch b = c//2, head-group hg = c%2, 8 heads each).
Each core computes the qkv projection for its (batch, head-group), causal
attention for its 8 heads, and a partial output projection. Host sums the
two partial y per batch (+ bout) and reassembles k, v.

Pipeline: per 512-token chunk ch, emit qkv projection for ch, then
attention for query chunk ch (keys 0..ch), then the y projection for ch.
The PE stream stays dense while the exp (ACT) stream for chunk ch overlaps
the projection matmuls of chunk ch+1.

Precision: qkv + output projection in float32r (tf32-grade); attention
internals (Q/K/V tiles, probabilities) in bf16 -- softmax averaging keeps
the y error ~1e-3; k/v outputs are staged from PSUM in full fp32.
"""

import numpy as np

B, T, C, H, D = 4, 2048, 1024, 16, 64
NHC = 8                # heads per core
HD = NHC * D           # 512 head-dims per core
SCALE = D ** -0.5
NCC = C // 128         # 8 contraction chunks for C
NM = HD // 128         # 4 head-pairs (m-tiles)
NJ = T // 512          # 4 chunks
NTT = T // 128         # 16 t-tiles
N_CORES = 8

_CACHE = {}


def _build_program(dbg=False):
    import concourse.bass as bass
    from concourse import bacc, tile, mybir

    FP = mybir.dt.float32
    MMD = mybir.dt.float32r
    BF = mybir.dt.bfloat16
    EXP = mybir.ActivationFunctionType.Exp

    nc = bacc.Bacc("TRN2", debug=False, target_bir_lowering=False)

    xT = nc.dram_tensor("xT", [C, T], MMD, kind="ExternalInput")
    wq = nc.dram_tensor("wq", [C, HD], MMD, kind="ExternalInput")
    wk = nc.dram_tensor("wk", [C, HD], MMD, kind="ExternalInput")
    wv = nc.dram_tensor("wv", [C, HD], MMD, kind="ExternalInput")
    wo = nc.dram_tensor("wo", [HD, C], MMD, kind="ExternalInput")
    bq = nc.dram_tensor("bq", [HD], FP, kind="ExternalInput")
    bk = nc.dram_tensor("bk", [HD], FP, kind="ExternalInput")
    bvb = nc.dram_tensor("bvb", [128, HD], FP, kind="ExternalInput")
    maskin = nc.dram_tensor("maskin", [128, 128], BF, kind="ExternalInput")
    y_o = nc.dram_tensor("y", [T, C], FP, kind="ExternalOutput")
    kT_o = nc.dram_tensor("kT", [HD, T], FP, kind="ExternalOutput")
    v_o = nc.dram_tensor("v", [T, HD], FP, kind="ExternalOutput")

    with tile.TileContext(nc) as tc:
        with (
            tc.tile_pool(name="persist", bufs=1) as pp,
            tc.tile_pool(name="pstp", bufs=2, space="PSUM") as stp,
            tc.tile_pool(name="opsp", bufs=2, space="PSUM") as ops,
            tc.tile_pool(name="xt", bufs=1) as xtp,
            tc.tile_pool(name="ptp", bufs=3) as ptp,
            tc.tile_pool(name="recp", bufs=1) as recp,
            tc.tile_pool(name="rbp", bufs=2) as rbp,
            tc.tile_pool(name="stg", bufs=2) as stg,
            tc.tile_pool(name="dscr", bufs=3, space="DRAM") as dscr,
        ):
            QT = [pp.tile([128, NM, 512], BF, tag=f"qt{i}", name=f"qt{i}")
                  for i in range(NJ)]
            KT = [pp.tile([128, NM, 512], BF, tag=f"kt{i}", name=f"kt{i}")
                  for i in range(NJ)]
            # V natural (bf16); 65th column per head = 1.0 so the O^T matmul
            # also accumulates the softmax denominator l as psum row 64.
            VN = [pp.tile([128, 4, NHC, 65], BF, tag=f"vn{i}", name=f"vn{i}")
                  for i in range(NJ)]
            OT = [pp.tile([128, NM, 512], MMD, tag=f"ot{i}", name=f"ot{i}")
                  for i in range(NJ)]
            wq_sb = pp.tile([128, NCC, HD], MMD)
            wk_sb = pp.tile([128, NCC, HD], MMD)
            wv_sb = pp.tile([128, NCC, HD], MMD)
            wo_sb = pp.tile([128, NM, C], MMD)
            bq_sb = pp.tile([128, NM], FP)
            bk_sb = pp.tile([128, NM], FP)
            bvb_sb = pp.tile([128, HD], FP)
            mask_sb = pp.tile([128, 128], BF)
            ones_f = pp.tile([128, 128], FP)

            nc.sync.dma_start(bq_sb[:], bq[:].rearrange("(m p) -> p m", p=128))
            nc.sync.dma_start(bk_sb[:], bk[:].rearrange("(m p) -> p m", p=128))
            nc.sync.dma_start(bvb_sb[:], bvb[:])
            nc.sync.dma_start(mask_sb[:], maskin[:])
            nc.gpsimd.memset(ones_f[:], 1.0)
            for i in range(NJ):
                nc.vector.tensor_copy(
                    VN[i][:, :, :, 64:65],
                    ones_f[:, 0:32].rearrange("p (a b c) -> p a b c",
                                              a=4, b=NHC))
            # weights on the scalar-engine DMA queue (ACT is idle early)
            nc.scalar.dma_start(
                wq_sb[:], wq[:].rearrange("(cc p) n -> p cc n", p=128))
            nc.scalar.dma_start(
                wk_sb[:], wk[:].rearrange("(cc p) n -> p cc n", p=128))
            nc.scalar.dma_start(
                wv_sb[:], wv[:].rearrange("(cc p) n -> p cc n", p=128))
            nc.scalar.dma_start(
                wo_sb[:], wo[:].rearrange("(m p) c -> p m c", p=128))

            for ch in range(NJ):
                ts = slice(ch * 512, ch * 512 + 512)
                # ---- qkv projection for chunk ch ----
                xt_t = xtp.tile([128, NCC, 512], MMD, tag="xt")
                nc.sync.dma_start(
                    xt_t[:],
                    xT[:].rearrange("(cc p) t -> p cc t", p=128)[:, :, ts])
                # Q
                for mp in range(2):
                    ps = stp.tile([128, 1024], FP, tag="pst")
                    for half in range(2):
                        m = 2 * mp + half
                        hs = slice(half * 512, half * 512 + 512)
                        for cc in range(NCC):
                            nc.tensor.matmul(
                                ps[:, hs],
                                wq_sb[:, cc, m * 128:m * 128 + 128],
                                xt_t[:, cc, :],
                                start=(cc == 0), stop=(cc == NCC - 1))
                        nc.vector.tensor_scalar_add(
                            QT[ch][:, m, :], ps[:, hs], bq_sb[:, m:m + 1])
                # K: bf16 tile for attention + fp32 staging for the k output
                for mp in range(2):
                    ps = stp.tile([128, 1024], FP, tag="pst")
                    kst = stg.tile([128, 1024], FP, tag="stg")
                    for half in range(2):
                        m = 2 * mp + half
                        hs = slice(half * 512, half * 512 + 512)
                        for cc in range(NCC):
                            nc.tensor.matmul(
                                ps[:, hs],
                                wk_sb[:, cc, m * 128:m * 128 + 128],
                                xt_t[:, cc, :],
                                start=(cc == 0), stop=(cc == NCC - 1))
                        nc.vector.tensor_scalar_add(
                            kst[:, hs], ps[:, hs], bk_sb[:, m:m + 1])
                        nc.vector.tensor_copy(KT[ch][:, m, :], kst[:, hs])
                    nc.sync.dma_start(
                        kT_o[mp * 256:mp * 256 + 256, ts].rearrange(
                            "(m p) t -> p m t", p=128),
                        kst[:].rearrange("p (m t) -> p m t", m=2))
                # V
                for sp in range(2):
                    ps = ops.tile([128, 1024], FP, tag="po")
                    vst = stg.tile([128, 1024], FP, tag="stg")
                    for half in range(2):
                        st = 2 * sp + half
                        hs = slice(half * 512, half * 512 + 512)
                        for cc in range(NCC):
                            nc.tensor.matmul(
                                ps[:, hs],
                                xt_t[:, cc, st * 128:st * 128 + 128],
                                wv_sb[:, cc, :],
                                start=(cc == 0), stop=(cc == NCC - 1))
                        nc.vector.tensor_add(vst[:, hs], ps[:, hs], bvb_sb[:])
                        nc.vector.tensor_copy(
                            VN[ch][:, st, :, 0:64],
                            vst[:, hs].rearrange("p (h d) -> p h d", h=NHC))
                        tt = ch * 4 + st
                        nc.sync.dma_start(
                            v_o[tt * 128:tt * 128 + 128, :].rearrange(
                                "p (h d) -> p h d", h=NHC),
                            vst[:, hs].rearrange("p (h d) -> p h d", h=NHC))

                # ---- attention for query chunk j = ch ----
                j = ch
                nkb = 4 * (j + 1)
                for hp in range(NM):
                    h0, h1 = 2 * hp, 2 * hp + 1
                    # po cols 0:512 even head, 512:1024 odd head;
                    # rows 0:64 = O^T, row 64 = softmax denominator
                    po = ops.tile([128, 1024], FP, tag="po")
                    for kb in range(nkb):
                        kc, ko = kb // 4, (kb % 4) * 128
                        pst = stp.tile([128, 1024], FP, tag="pst")
                        nc.tensor.matmul(
                            pst[:, 0:512],
                            KT[kc][0:64, hp, ko:ko + 128],
                            QT[j][0:64, hp, :])
                        nc.tensor.matmul(
                            pst[:, 512:1024],
                            KT[kc][64:128, hp, ko:ko + 128],
                            QT[j][64:128, hp, :])
                        pt = ptp.tile([128, 1024], BF, tag="pt")
                        nc.scalar.activation(pt[:], pst[:], EXP, scale=SCALE)
                        rband = kb - 4 * j
                        lo = 0
                        if rband >= 0:
                            # band block: cols [0,128r) fully masked ->
                            # skipped in the O matmul; [128r,128r+128)
                            # get the triangular mask
                            lo = 128 * rband
                            for half in (0, 512):
                                nc.vector.tensor_mul(
                                    pt[:, half + lo:half + lo + 128],
                                    pt[:, half + lo:half + lo + 128],
                                    mask_sb[:])
                        mmkw = dict(start=(kb == 0), stop=(kb == nkb - 1))
                        nc.tensor.matmul(
                            po[0:65, lo:512],
                            VN[kc][:, kb % 4, h0, :],
                            pt[:, lo:512], **mmkw)
                        nc.tensor.matmul(
                            po[0:65, 512 + lo:1024],
                            VN[kc][:, kb % 4, h1, :],
                            pt[:, 512 + lo:1024], **mmkw)
                    # normalize: 1/l then broadcast across partitions via a
                    # DRAM bounce (SBUF APs can't partition-broadcast)
                    ls = recp.tile([128, 2048], FP, tag="ls")
                    nc.vector.tensor_copy(ls[0:1, 0:1024], po[64:65, :])
                    nc.vector.reciprocal_approx_fast(
                        ls[0:1, 1024:2048], ls[0:1, 0:1024])
                    scr = dscr.tile([1, 1024], FP, tag="scr")
                    nc.gpsimd.dma_start(scr[:], ls[0:1, 1024:2048])
                    rb = rbp.tile([128, 512], FP, tag="rb")
                    nc.gpsimd.dma_start(
                        rb[0:64, :], scr[0:1, 0:512].to_broadcast((64, 512)))
                    nc.gpsimd.dma_start(
                        rb[64:128, :],
                        scr[0:1, 512:1024].to_broadcast((64, 512)))
                    nc.vector.tensor_mul(
                        OT[j][0:64, hp, :], po[0:64, 0:512], rb[0:64, :])
                    nc.vector.tensor_mul(
                        OT[j][64:128, hp, :], po[0:64, 512:1024],
                        rb[64:128, :])

                # ---- y projection for chunk ch ----
                for st in range(4):
                    tt = ch * 4 + st
                    ps = stp.tile([128, 1024], FP, tag="pst")
                    for cc2 in range(2):
                        cs = slice(cc2 * 512, cc2 * 512 + 512)
                        for hp in range(NM):
                            nc.tensor.matmul(
                                ps[:, cs],
                                OT[ch][:, hp, st * 128:st * 128 + 128],
                                wo_sb[:, hp, cs],
                                start=(hp == 0), stop=(hp == NM - 1))
                    yt = stg.tile([128, 1024], FP, tag="stg")
                    nc.vector.tensor_copy(yt[:], ps[:])
                    nc.sync.dma_start(y_o[tt * 128:tt * 128 + 128, :], yt[:])

    nc.compile()
    return nc


def _get_program(dbg=False):
    key = ("nc", dbg)
    if key not in _CACHE:
        _CACHE[key] = _build_program(dbg)
    return _CACHE[key]


def _make_mask():
    # (128,128) lower-triangular 0/1: mask[p, g] = 1.0 iff g >= p
    p = np.arange(128)[:, None]
    g = np.arange(128)[None, :]
    return (g >= p)


def _shard_inputs(x, Wqkv, bqkv, Wout):
    import ml_dtypes
    mask = _make_mask().astype(ml_dtypes.bfloat16)
    xT_b = [np.ascontiguousarray(x[b].T) for b in range(B)]
    in_maps = []
    for c in range(N_CORES):
        b, hg = c // 2, c % 2
        bv = bqkv[2 * C + hg * HD: 2 * C + hg * HD + HD]
        in_maps.append({
            "xT": xT_b[b],
            "wq": np.ascontiguousarray(Wqkv[:, hg * HD: hg * HD + HD]),
            "wk": np.ascontiguousarray(Wqkv[:, C + hg * HD: C + hg * HD + HD]),
            "wv": np.ascontiguousarray(Wqkv[:, 2 * C + hg * HD: 2 * C + hg * HD + HD]),
            "wo": np.ascontiguousarray(Wout[hg * HD: hg * HD + HD, :]),
            "bq": np.ascontiguousarray(bqkv[hg * HD: hg * HD + HD]),
            "bk": np.ascontiguousarray(bqkv[C + hg * HD: C + hg * HD + HD]),
            "bvb": np.ascontiguousarray(np.tile(bv, (128, 1))),
            "maskin": mask,
        })
    return in_maps


def _unshard(results, bqkv, bout):
    y = np.empty((B, T, C), dtype=np.float32)
    k = np.empty((B, H, T, D), dtype=np.float32)
    v = np.empty((B, H, T, D), dtype=np.float32)
    for c in range(N_CORES):
        b, hg = c // 2, c % 2
        res = results[c]
        hsl = slice(hg * NHC, hg * NHC + NHC)
        k[b, hsl] = res["kT"].reshape(NHC, D, T).transpose(0, 2, 1)
        v[b, hsl] = res["v"].reshape(T, NHC, D).transpose(1, 0, 2)
    for b in range(B):
        y[b] = results[2 * b]["y"] + results[2 * b + 1]["y"] + bout
    return y, k, v


LAST_EXEC_NS = None
LAST_RESULT = None
PROFILE = False
PROFILE_DIR = None


def kernel(x, Wqkv, bqkv, Wout, bout):
    global LAST_EXEC_NS, LAST_RESULT
    from concourse.bass_utils import run_bass_kernel_spmd

    x = np.asarray(x, dtype=np.float32)
    Wqkv = np.asarray(Wqkv, dtype=np.float32)
    bqkv = np.asarray(bqkv, dtype=np.float32)
    Wout = np.asarray(Wout, dtype=np.float32)
    bout = np.asarray(bout, dtype=np.float32)

    nc = _get_program()
    in_maps = _shard_inputs(x, Wqkv, bqkv, Wout)
    try:
        res = run_bass_kernel_spmd(
            nc, in_maps, list(range(N_CORES)), trace=PROFILE,
            tmpdir=PROFILE_DIR)
    except Exception:
        # transient device errors (e.g. NRT_EXEC_UNIT_UNRECOVERABLE) clear
        # on a retry
        res = run_bass_kernel_spmd(
            nc, in_maps, list(range(N_CORES)), trace=PROFILE,
            tmpdir=PROFILE_DIR)
    LAST_EXEC_NS = res.exec_time_ns
    LAST_RESULT = res
    return _unshard(res.results, bqkv, bout)
